# revision 47
# baseline (speedup 1.0000x reference)
"""Multi-head attention kernel for Trainium2, SPMD across 8 NeuronCores.

Problem: x[8,16,256,384] -> attention(8 heads, head_dim 64) -> [8,16,256,384]
Sharding: data-parallel over batch b (1 batch element per core, weights
replicated). Each core processes 16 independent slices of [256 tokens, 384],
handled in pairs ("superslices") so the QKV matmuls stream N=512.

Per-slice dataflow (activations kept feature-major, i.e. transposed):
  xT[d,t]   = DMA-transpose load of x slice         [384, 256] (bf16)
  qkT[e,t]  = w_qkv[:,e].T @ xT  (e in 0..1023)     q^T,k^T feature-major
  v[t,e]    = xT.T @ w_qkv[:, 1024:1536]            natural layout
  sT[j,i]   = k_h^T.T @ q_h^T   (per head, K=64; head pairs row-tiled)
  pT        = exp(sT / 8)                           [j, 2 heads x i]
  rowsum    = ones.T @ pT                           [1, 512] matmul
  o         = v_h.T @ pT  (head pair col-tiled into one [128,256] PSUM)
  oT        = o * broadcast(1/rowsum)               one mul per head pair
  out[t,:]  = oT.T @ w_out + b_out (bias via K=1 ones matmul)
"""

import sys
import types

sys.path.insert(0, "/opt/trn_rl_repo")

import numpy as np

import concourse.bass as bass
import concourse.bacc as bacc
import concourse.mybir as mybir
import concourse.tile as tile
from concourse.bass_utils import run_bass_kernel_spmd

N_CORES = 8
B, P, N, D = 8, 16, 256, 384
H, HD = 8, 64
INNER = H * HD  # 512
SCALE = HD ** -0.5
F32 = mybir.dt.float32

MM_MODE = "bf16"  # "bf16" | "f32r" | "f32"


def _mdt(mm_mode):
    return {"bf16": mybir.dt.bfloat16,
            "f32r": mybir.dt.float32r,
            "f32": F32}[mm_mode]


def _np_mdt(mm_mode):
    if mm_mode == "bf16":
        import ml_dtypes
        return ml_dtypes.bfloat16
    return np.float32


def _register_ntff_hook():
    """Make trace=True work under axon when antenv.axon_hooks is absent."""
    if "antenv.axon_hooks" in sys.modules:
        return
    try:
        from trn_agent_boot.trn_boot import _ntff_profile_via_ctypes
    except ImportError:
        return
    hook = _ntff_profile_via_ctypes("/opt/axon/libaxon_pjrt.so")
    mod = types.ModuleType("antenv.axon_hooks")
    mod.get_axon_ntff_profile_hook = lambda: hook
    sys.modules["antenv.axon_hooks"] = mod


def build(mm_mode=MM_MODE):
    nc = bacc.Bacc("TRN2", target_bir_lowering=False, debug=False,
                   num_devices=N_CORES)
    MDT = _mdt(mm_mode)
    x_ext = nc.declare_dram_parameter("x", [P, N, D], MDT, isOutput=False)
    wq_ext = nc.declare_dram_parameter("w_qkv", [D, 3 * INNER], MDT,
                                       isOutput=False)
    wo_ext = nc.declare_dram_parameter("w_out", [INNER, D], MDT,
                                       isOutput=False)
    bo_ext = nc.declare_dram_parameter("b_out", [D], MDT, isOutput=False)
    out_ext = nc.declare_dram_parameter("out", [P, N, D], F32, isOutput=True)

    Exp = mybir.ActivationFunctionType.Exp
    memset_dt = F32 if mm_mode != "bf16" else MDT

    with tile.TileContext(nc) as tc:
        with (
            tc.tile_pool(name="const", bufs=1) as const,
            tc.tile_pool(name="xt", bufs=3) as xt_pool,
            tc.tile_pool(name="qk", bufs=3) as qk_pool,
            tc.tile_pool(name="vp", bufs=3) as v_pool,
            tc.tile_pool(name="pt", bufs=8) as p_pool,
            tc.tile_pool(name="ot", bufs=6) as ot_pool,
            tc.tile_pool(name="ob", bufs=4) as ob_pool,
            tc.tile_pool(name="rs", bufs=8) as rs_pool,
            tc.tile_pool(name="bc", bufs=8) as bc_pool,
            tc.tile_pool(name="mmps", bufs=3, space="PSUM") as mm_ps,
            tc.tile_pool(name="sps", bufs=2, space="PSUM") as s_ps,
            tc.tile_pool(name="ops", bufs=3, space="PSUM") as o_ps,

        ):
            # ---- constants (loaded once); w split per kc chunk so the
            # first matmuls wait only on their own chunk ----
            w_sbs = []
            for kc in range(3):
                wt = const.tile([128, 1536], MDT, tag=f"w_sb{kc}")
                nc.sync.dma_start(wt[:], wq_ext.ap()[kc * 128:(kc + 1) * 128, :])
                w_sbs.append(wt)
            wo_sb = const.tile([128, 4 * 384], MDT, tag="wo_sb")
            for kc in range(4):
                nc.sync.dma_start(wo_sb[:, kc * 384:(kc + 1) * 384],
                                  wo_ext.ap()[kc * 128:(kc + 1) * 128, :])
            bt_sb = const.tile([1, 384], MDT, tag="bt_sb")
            nc.sync.dma_start(bt_sb[:], bo_ext.ap().unsqueeze(0))
            on_sb = const.tile([128, 128], MDT, tag="on_sb")
            nc.gpsimd.memset(on_sb[:], 1.0)

            # m-chunk order: interleave q and k chunks so head-pair c has
            # its q (m=c) and k (m=4+c) chunks available early.
            m_order = [0, 4, 1, 5, 2, 6, 3, 7]

            for u in range(P // 2):  # superslice of 2 token slices
                # ---- xT via DMA transpose: [256,128] dram -> [128,256],
                # one tile per kc chunk for fine-grained dependencies ----
                xts = []
                for kc in range(3):
                    xtk = xt_pool.tile([128, 512], MDT, tag=f"xtk{kc % 3}")
                    xts.append(xtk)
                for kc in range(3):
                    for a in range(2):
                        nc.sync.dma_start(
                            xts[kc][:, a * 256:(a + 1) * 256],
                            x_ext.ap()[2 * u + a, :, kc * 128:(kc + 1) * 128],
                            transpose=True)

                # ---- qkT chunks m (features m*128..m*128+127) ----
                qk = qk_pool.tile([128, 8 * 512], MDT, tag="qk")
                for mi, m in enumerate(m_order):
                    ps = mm_ps.tile([128, 512], F32, tag="mmps")
                    for kc in range(3):
                        nc.tensor.matmul(
                            ps[:],
                            w_sbs[kc][:, m * 128:(m + 1) * 128],
                            xts[kc][:],
                            start=(kc == 0), stop=(kc == 2))
                    if mi % 2 == 0:
                        nc.scalar.copy(qk[:, m * 512:(m + 1) * 512], ps[:])
                    else:
                        nc.vector.tensor_copy(qk[:, m * 512:(m + 1) * 512],
                                              ps[:])

                # ---- v: per (slice,tok-chunk) 8 blocks x 128 cols:
                # col 0 = ones (rowsum row 0), cols 64:128 = v features ----
                v = v_pool.tile([128, 4 * 1024], MDT, tag="v")
                ones_cols = v[:].rearrange("p (a c) -> p a c", c=128)[:, :, 0:1]
                nc.gpsimd.memset(ones_cols.bitcast(memset_dt), 1.0)
                for a in range(2):
                    for t in range(2):
                        ps = mm_ps.tile([128, 512], F32, tag="mmps")
                        for kc in range(3):
                            nc.tensor.matmul(
                                ps[:],
                                xts[kc][:, a * 256 + t * 128:
                                        a * 256 + (t + 1) * 128],
                                w_sbs[kc][:, 1024:1536],
                                start=(kc == 0), stop=(kc == 2))
                        dst = v[:, (a * 2 + t) * 1024:
                                (a * 2 + t) * 1024 + 1024]
                        dst = dst.rearrange("p (h c) -> p h c",
                                            c=128)[:, :, 64:128]
                        if t == 0:
                            nc.scalar.copy(
                                dst,
                                ps[:].rearrange("p (h c) -> p h c", c=64))
                        else:
                            nc.vector.tensor_copy(
                                dst,
                                ps[:].rearrange("p (h c) -> p h c", c=64))

                # ---- attention: head pairs (2c, 2c+1), both slices
                # interleaved for independent dependency chains ----
                ots = []
                for a in range(2):
                    ot = ot_pool.tile([128, 4 * 256], MDT, tag=f"ot{a}")
                    ots.append(ot)
                for c in range(4):
                    for a in range(2):
                        ot = ots[a]
                        pts = []
                        for jc in range(2):
                            pt = p_pool.tile([128, 512], MDT, tag="pt")
                            for e in range(2):
                                sps = s_ps.tile([128, 256], F32, tag="sps")
                                nc.tensor.matmul(
                                    sps[:],
                                    qk[e * 64:e * 64 + 64,
                                       (4 + c) * 512 + a * 256 + jc * 128:
                                       (4 + c) * 512 + a * 256 + (jc + 1) * 128],
                                    qk[e * 64:e * 64 + 64,
                                       c * 512 + a * 256: c * 512 + (a + 1) * 256],
                                    start=True, stop=True,
                                    tile_position=(e * 64, 0))
                                nc.scalar.activation(
                                    pt[:, e * 256:(e + 1) * 256], sps[:], Exp,
                                    scale=SCALE)
                            pts.append(pt)
                        ops = o_ps.tile([128, 512], F32, tag="ops")
                        for e in range(2):
                            h = 2 * c + e
                            for jc in range(2):
                                nc.tensor.matmul(
                                    ops[:, e * 256:(e + 1) * 256],
                                    v[:, (a * 2 + jc) * 1024 + h * 128:
                                      (a * 2 + jc) * 1024 + (h + 1) * 128],
                                    pts[jc][:, e * 256:(e + 1) * 256],
                                    start=(jc == 0), stop=(jc == 1))
                        rs = rs_pool.tile([1, 512], F32, tag="rs")
                        nc.vector.reciprocal_approx_fast(rs[:], ops[0:1, :])
                        bc = bc_pool.tile([64, 512], F32, tag="bc")
                        nc.gpsimd.partition_broadcast(bc[:], rs[0:1, :])
                        for e in range(2):
                            nc.vector.tensor_mul(
                                ot[e * 64:(e + 1) * 64,
                                   c * 256:(c + 1) * 256],
                                ops[64:128, e * 256:(e + 1) * 256],
                                bc[:, e * 256:(e + 1) * 256])

                # ---- output projection; bias via K=1 ones matmul ----
                for a in range(2):
                    ot = ots[a]
                    for t in range(2):
                        fps = mm_ps.tile([128, 512], F32, tag="mmps")
                        for kc in range(4):
                            nc.tensor.matmul(
                                fps[:, 0:384],
                                ot[:, kc * 256 + t * 128:
                                   kc * 256 + (t + 1) * 128],
                                wo_sb[:, kc * 384:(kc + 1) * 384],
                                start=(kc == 0), stop=False)
                        nc.tensor.matmul(
                            fps[:, 0:384], on_sb[0:1, 0:128], bt_sb[:],
                            start=False, stop=True)
                        ob = ob_pool.tile([128, 384], F32, tag="ob")
                        nc.scalar.copy(ob[:], fps[:, 0:384])
                        nc.scalar.dma_start(
                            out_ext.ap()[2 * u + a, t * 128:(t + 1) * 128, :],
                            ob[:])
    nc.compile()
    return nc


_CACHE = {}


def _get_nc(mm_mode=MM_MODE):
    if mm_mode not in _CACHE:
        _CACHE[mm_mode] = build(mm_mode)
    return _CACHE[mm_mode]


def _in_maps(inputs, mm_mode=MM_MODE):
    ndt = _np_mdt(mm_mode)
    x = np.asarray(inputs["x"]).astype(ndt)
    w_qkv = np.asarray(inputs["w_qkv"]).astype(ndt)
    w_out = np.asarray(inputs["w_out"]).astype(ndt)
    b_out = np.asarray(inputs["b_out"]).astype(ndt)
    return [
        {"x": np.ascontiguousarray(x[i]), "w_qkv": w_qkv, "w_out": w_out,
         "b_out": b_out}
        for i in range(N_CORES)
    ]


def run(inputs, trace=False, mm_mode=MM_MODE):
    """Returns (output [8,16,256,384], exec_time_ns or None)."""
    if trace:
        _register_ntff_hook()
    nc = _get_nc(mm_mode)
    res = run_bass_kernel_spmd(nc, _in_maps(inputs, mm_mode),
                               core_ids=list(range(N_CORES)), trace=trace)
    out = np.stack([res.results[i]["out"] for i in range(N_CORES)], axis=0)
    return out, res.exec_time_ns


def kernel(**inputs) -> np.ndarray:
    out, _ = run(inputs, trace=False)
    return out


# revision 53
# speedup vs baseline: 1.0064x; 1.0064x over previous
"""Multi-head attention kernel for Trainium2, SPMD across 8 NeuronCores.

Problem: x[8,16,256,384] -> attention(8 heads, head_dim 64) -> [8,16,256,384]
Sharding: data-parallel over batch b (1 batch element per core, weights
replicated). Each core processes 16 independent slices of [256 tokens, 384],
handled in pairs ("superslices") so the QKV matmuls stream N=512.

Per-slice dataflow (activations kept feature-major, i.e. transposed):
  xT[d,t]   = DMA-transpose load of x slice         [384, 256] (bf16)
  qkT[e,t]  = w_qkv[:,e].T @ xT  (e in 0..1023)     q^T,k^T feature-major
  v[t,e]    = xT.T @ w_qkv[:, 1024:1536]            natural layout
  sT[j,i]   = k_h^T.T @ q_h^T   (per head, K=64; head pairs row-tiled)
  pT        = exp(sT / 8)                           [j, 2 heads x i]
  rowsum    = ones.T @ pT                           [1, 512] matmul
  o         = v_h.T @ pT  (head pair col-tiled into one [128,256] PSUM)
  oT        = o * broadcast(1/rowsum)               one mul per head pair
  out[t,:]  = oT.T @ w_out + b_out (bias via K=1 ones matmul)
"""

import sys
import types

sys.path.insert(0, "/opt/trn_rl_repo")

import numpy as np

import concourse.bass as bass
import concourse.bacc as bacc
import concourse.mybir as mybir
import concourse.tile as tile
from concourse.bass_utils import run_bass_kernel_spmd

N_CORES = 8
B, P, N, D = 8, 16, 256, 384
H, HD = 8, 64
INNER = H * HD  # 512
SCALE = HD ** -0.5
F32 = mybir.dt.float32

MM_MODE = "bf16"  # "bf16" | "f32r" | "f32"


def _mdt(mm_mode):
    return {"bf16": mybir.dt.bfloat16,
            "f32r": mybir.dt.float32r,
            "f32": F32}[mm_mode]


def _np_mdt(mm_mode):
    if mm_mode == "bf16":
        import ml_dtypes
        return ml_dtypes.bfloat16
    return np.float32


def _register_ntff_hook():
    """Make trace=True work under axon when antenv.axon_hooks is absent."""
    if "antenv.axon_hooks" in sys.modules:
        return
    try:
        from trn_agent_boot.trn_boot import _ntff_profile_via_ctypes
    except ImportError:
        return
    hook = _ntff_profile_via_ctypes("/opt/axon/libaxon_pjrt.so")
    mod = types.ModuleType("antenv.axon_hooks")
    mod.get_axon_ntff_profile_hook = lambda: hook
    sys.modules["antenv.axon_hooks"] = mod


def build(mm_mode=MM_MODE):
    nc = bacc.Bacc("TRN2", target_bir_lowering=False, debug=False,
                   num_devices=N_CORES)
    MDT = _mdt(mm_mode)
    x_ext = nc.declare_dram_parameter("x", [P, N, D], MDT, isOutput=False)
    wq_ext = nc.declare_dram_parameter("w_qkv", [D, 3 * INNER], MDT,
                                       isOutput=False)
    wo_ext = nc.declare_dram_parameter("w_out", [INNER, D], MDT,
                                       isOutput=False)
    bo_ext = nc.declare_dram_parameter("b_out", [D], MDT, isOutput=False)
    out_ext = nc.declare_dram_parameter("out", [P, N, D], F32, isOutput=True)

    Exp = mybir.ActivationFunctionType.Exp
    memset_dt = F32 if mm_mode != "bf16" else MDT

    with tile.TileContext(nc) as tc:
        with (
            tc.tile_pool(name="const", bufs=1) as const,
            tc.tile_pool(name="xt", bufs=3) as xt_pool,
            tc.tile_pool(name="qk", bufs=3) as qk_pool,
            tc.tile_pool(name="vp", bufs=3) as v_pool,
            tc.tile_pool(name="pt", bufs=12) as p_pool,
            tc.tile_pool(name="ot", bufs=6) as ot_pool,
            tc.tile_pool(name="ob", bufs=6) as ob_pool,
            tc.tile_pool(name="rs", bufs=8) as rs_pool,
            tc.tile_pool(name="bc", bufs=8) as bc_pool,
            tc.tile_pool(name="mmps", bufs=3, space="PSUM") as mm_ps,
            tc.tile_pool(name="sps", bufs=2, space="PSUM") as s_ps,
            tc.tile_pool(name="ops", bufs=3, space="PSUM") as o_ps,

        ):
            # ---- constants (loaded once) ----
            w_sb = const.tile([128, 3 * 1536], MDT, tag="w_sb")
            for kc in range(3):
                nc.sync.dma_start(w_sb[:, kc * 1536:(kc + 1) * 1536],
                                  wq_ext.ap()[kc * 128:(kc + 1) * 128, :])
            wo_sb = const.tile([128, 4 * 384], MDT, tag="wo_sb")
            for kc in range(4):
                nc.sync.dma_start(wo_sb[:, kc * 384:(kc + 1) * 384],
                                  wo_ext.ap()[kc * 128:(kc + 1) * 128, :])
            bt_sb = const.tile([1, 384], MDT, tag="bt_sb")
            nc.sync.dma_start(bt_sb[:], bo_ext.ap().unsqueeze(0))
            on_sb = const.tile([128, 128], MDT, tag="on_sb")
            nc.gpsimd.memset(on_sb[:], 1.0)

            # m-chunk order: interleave q and k chunks so head-pair c has
            # its q (m=c) and k (m=4+c) chunks available early.
            m_order = [0, 4, 1, 5, 2, 6, 3, 7]

            for u in range(P // 2):  # superslice of 2 token slices
                # ---- xT via DMA transpose: [256,128] dram -> [128,256] ----
                xt = xt_pool.tile([128, 3 * 512], MDT, tag="xt")
                for a in range(2):
                    for kc in range(3):
                        nc.sync.dma_start(
                            xt[:, kc * 512 + a * 256: kc * 512 + (a + 1) * 256],
                            x_ext.ap()[2 * u + a, :, kc * 128:(kc + 1) * 128],
                            transpose=True)

                # ---- qkT chunks m (features m*128..m*128+127) ----
                qk = qk_pool.tile([128, 8 * 512], MDT, tag="qk")
                for mi, m in enumerate(m_order):
                    ps = mm_ps.tile([128, 512], F32, tag="mmps")
                    for kc in range(3):
                        nc.tensor.matmul(
                            ps[:],
                            w_sb[:, kc * 1536 + m * 128:
                                 kc * 1536 + (m + 1) * 128],
                            xt[:, kc * 512:(kc + 1) * 512],
                            start=(kc == 0), stop=(kc == 2))
                    if mi % 2 == 0:
                        nc.scalar.copy(qk[:, m * 512:(m + 1) * 512], ps[:])
                    else:
                        nc.vector.tensor_copy(qk[:, m * 512:(m + 1) * 512],
                                              ps[:])

                # ---- v: per (slice,tok-chunk) 8 blocks x 128 cols:
                # col 0 = ones (rowsum row 0), cols 64:128 = v features ----
                v = v_pool.tile([128, 4 * 1024], MDT, tag="v")
                ones_cols = v[:].rearrange("p (a c) -> p a c", c=128)[:, :, 0:1]
                nc.gpsimd.memset(ones_cols.bitcast(memset_dt), 1.0)
                for a in range(2):
                    for t in range(2):
                        ps = mm_ps.tile([128, 512], F32, tag="mmps")
                        for kc in range(3):
                            nc.tensor.matmul(
                                ps[:],
                                xt[:, kc * 512 + a * 256 + t * 128:
                                   kc * 512 + a * 256 + (t + 1) * 128],
                                w_sb[:, kc * 1536 + 1024: kc * 1536 + 1536],
                                start=(kc == 0), stop=(kc == 2))
                        dst = v[:, (a * 2 + t) * 1024:
                                (a * 2 + t) * 1024 + 1024]
                        dst = dst.rearrange("p (h c) -> p h c",
                                            c=128)[:, :, 64:128]
                        if t == 0:
                            nc.scalar.copy(
                                dst,
                                ps[:].rearrange("p (h c) -> p h c", c=64))
                        else:
                            nc.vector.tensor_copy(
                                dst,
                                ps[:].rearrange("p (h c) -> p h c", c=64))

                # ---- attention: head pairs (2c, 2c+1), both slices
                # interleaved for independent dependency chains ----
                ots = []
                for a in range(2):
                    ot = ot_pool.tile([128, 4 * 256], MDT, tag=f"ot{a}")
                    ots.append(ot)
                for c in range(4):
                    for a in range(2):
                        ot = ots[a]
                        pts = []
                        for jc in range(2):
                            pt = p_pool.tile([128, 512], MDT, tag="pt")
                            for e in range(2):
                                sps = s_ps.tile([128, 256], F32, tag="sps")
                                nc.tensor.matmul(
                                    sps[:],
                                    qk[e * 64:e * 64 + 64,
                                       (4 + c) * 512 + a * 256 + jc * 128:
                                       (4 + c) * 512 + a * 256 + (jc + 1) * 128],
                                    qk[e * 64:e * 64 + 64,
                                       c * 512 + a * 256: c * 512 + (a + 1) * 256],
                                    start=True, stop=True,
                                    tile_position=(e * 64, 0))
                                nc.scalar.activation(
                                    pt[:, e * 256:(e + 1) * 256], sps[:], Exp,
                                    scale=SCALE)
                            pts.append(pt)
                        ops = o_ps.tile([128, 512], F32, tag="ops")
                        for e in range(2):
                            h = 2 * c + e
                            for jc in range(2):
                                nc.tensor.matmul(
                                    ops[:, e * 256:(e + 1) * 256],
                                    v[:, (a * 2 + jc) * 1024 + h * 128:
                                      (a * 2 + jc) * 1024 + (h + 1) * 128],
                                    pts[jc][:, e * 256:(e + 1) * 256],
                                    start=(jc == 0), stop=(jc == 1))
                        rs = rs_pool.tile([1, 512], F32, tag="rs")
                        nc.vector.reciprocal_approx_fast(rs[:], ops[0:1, :])
                        bc = bc_pool.tile([64, 512], F32, tag="bc")
                        nc.gpsimd.partition_broadcast(bc[:], rs[0:1, :])
                        for e in range(2):
                            nc.vector.tensor_mul(
                                ot[e * 64:(e + 1) * 64,
                                   c * 256:(c + 1) * 256],
                                ops[64:128, e * 256:(e + 1) * 256],
                                bc[:, e * 256:(e + 1) * 256])

                # ---- output projection; bias via K=1 ones matmul ----
                for a in range(2):
                    ot = ots[a]
                    for t in range(2):
                        fps = mm_ps.tile([128, 512], F32, tag="mmps")
                        for kc in range(4):
                            nc.tensor.matmul(
                                fps[:, 0:384],
                                ot[:, kc * 256 + t * 128:
                                   kc * 256 + (t + 1) * 128],
                                wo_sb[:, kc * 384:(kc + 1) * 384],
                                start=(kc == 0), stop=False)
                        nc.tensor.matmul(
                            fps[:, 0:384], on_sb[0:1, 0:128], bt_sb[:],
                            start=False, stop=True)
                        ob = ob_pool.tile([128, 384], F32, tag="ob")
                        nc.scalar.copy(ob[:], fps[:, 0:384])
                        nc.scalar.dma_start(
                            out_ext.ap()[2 * u + a, t * 128:(t + 1) * 128, :],
                            ob[:])
    nc.compile()
    return nc


_CACHE = {}


def _get_nc(mm_mode=MM_MODE):
    if mm_mode not in _CACHE:
        _CACHE[mm_mode] = build(mm_mode)
    return _CACHE[mm_mode]


def _in_maps(inputs, mm_mode=MM_MODE):
    ndt = _np_mdt(mm_mode)
    x = np.asarray(inputs["x"]).astype(ndt)
    w_qkv = np.asarray(inputs["w_qkv"]).astype(ndt)
    w_out = np.asarray(inputs["w_out"]).astype(ndt)
    b_out = np.asarray(inputs["b_out"]).astype(ndt)
    return [
        {"x": np.ascontiguousarray(x[i]), "w_qkv": w_qkv, "w_out": w_out,
         "b_out": b_out}
        for i in range(N_CORES)
    ]


def run(inputs, trace=False, mm_mode=MM_MODE):
    """Returns (output [8,16,256,384], exec_time_ns or None)."""
    if trace:
        _register_ntff_hook()
    nc = _get_nc(mm_mode)
    res = run_bass_kernel_spmd(nc, _in_maps(inputs, mm_mode),
                               core_ids=list(range(N_CORES)), trace=trace)
    out = np.stack([res.results[i]["out"] for i in range(N_CORES)], axis=0)
    return out, res.exec_time_ns


def kernel(**inputs) -> np.ndarray:
    out, _ = run(inputs, trace=False)
    return out
